# revision 51
# baseline (speedup 1.0000x reference)
"""Trainium2 Bass kernel: depthwise 19x19 Gaussian blur (sigma learnable scalar).

Math: the normalized 2D Gaussian kernel is exactly separable, K2 = outer(t, t)
with t = g / sum(g), g[i] = exp(-(i-9)^2 / (2 sigma^2)).  Each 1D conv (SAME,
zero pad) is a banded-matrix product.  On the TensorEngine, a matmul
out[m,n] = sum_k lhsT[k,m] * rhs[k,n] with lhsT = image chunk (stationary) and
rhs = banded matrix A (A[h,h'] = t[h-h'+9]) computes the vertical conv AND
transposes, so two chained passes land back in the original orientation with
zero explicit transposes:
  pass1: y1T[w,h'] = sum_h img[h,w]  * A[h,h']   (img tiles stationary)
  pass2: out[h',w']= sum_w y1T[w,h'] * A[w,w']   (y1T tiles stationary)
Bands are trimmed per 128-row k-chunk so each matmul streams only ~140-152
columns; the 4 chunk contributions accumulate into one PSUM bank per output
chunk (start=True only on the first).

All HBM I/O and matmul operands are fp16 (PSUM accumulation stays fp32):
fp16 matmuls stream 1 cycle/row vs fp32's 4, and the DMA bytes halve.  The
output tolerance (rel err < 2e-2) leaves ample room (~4e-4 realized).
PSUM->SBUF copies alternate between DVE and Activation (2-bank tiles, one
copy per two matmul groups); the input stream rides the GpSimd SWDGE queue
and the output stream the SP HWDGE queue.  x/y use a partition-major DRAM
layout (host permute) for 2KB DMA packets; A is stored band-compact.

Sharding: pure data parallel, 2 batches (32 images of 512x512) per core
across 8 cores.
"""

import sys

for _p in ("/opt/trn_rl_repo", "/root/.axon_site/_ro/trn_rl_repo"):
    if _p not in sys.path:
        sys.path.append(_p)

import numpy as np

H = 512
W = 512
KS = 19
HALF = KS // 2
CH = 16
BATCH = 16
NCORES = 8
B_PER_CORE = BATCH // NCORES          # 2
IMGS = B_PER_CORE * CH                # 32 images per core
P = 128
NCH = H // P                          # 4 chunks of 128 rows
# Per k-chunk c the nonzero band of A covers columns [128c-9, 128c+136];
# widened to 8B-aligned boundaries (multiples of 4 fp16 elems), clipped
# to [0, 512).
NR = [(0, 140), (116, 268), (244, 396), (372, 512)]
WB = max(n1 - n0 for n0, n1 in NR)            # 152: compact band storage


def _taps(sigma: float) -> np.ndarray:
    coords = np.arange(-HALF, HALF + 1, dtype=np.float64)
    g = np.exp(-(coords ** 2) / (2.0 * float(sigma) ** 2))
    return g / g.sum()


def band_matrix(sigma: float) -> np.ndarray:
    """A[i, j] = t[i - j + HALF] for |i-j| <= HALF, else 0.  (512, 512)."""
    t = _taps(sigma)
    A = np.zeros((H, H), np.float64)
    idx = np.arange(H)
    for d in range(-HALF, HALF + 1):
        sel = idx[(idx + d >= 0) & (idx + d < H)]
        A[sel, sel + d] = t[HALF - d]
    return A.astype(np.float16)


_NC_CACHE = {}


def _build_nc():
    if "nc" in _NC_CACHE:
        return _NC_CACHE["nc"]
    from concourse import bacc, tile, mybir

    f16 = mybir.dt.float16
    f32 = mybir.dt.float32
    nc = bacc.Bacc(None)
    # x and y live in DRAM partition-major ([img, p, c, w] with h = c*128+p,
    # permuted on the host): every SBUF partition's 4KB is contiguous in
    # DRAM, so the DGE emits large packets instead of 1KB row packets.
    x = nc.declare_dram_parameter("x", [IMGS, P, NCH, W], f16, isOutput=False)
    # A stored band-compact ([p, c, 0:n1-n0] = A[c*128+p, n0:n1]): 156KB
    # instead of 524KB, so it lands in SBUF well before the first image.
    a = nc.declare_dram_parameter("a", [P, NCH, WB], f16, isOutput=False)
    y = nc.declare_dram_parameter("y", [IMGS, P, NCH, W], f16, isOutput=True)

    xr = x                                          # [IMGS, 128, 4, 512]

    with tile.TileContext(nc) as tc:
        with (
            tc.tile_pool(name="aco", bufs=1) as a_pool,
            tc.tile_pool(name="img", bufs=6) as img_pool,
            tc.tile_pool(name="mid", bufs=4) as mid_pool,
            tc.tile_pool(name="ost", bufs=6) as out_pool,
            tc.tile_pool(name="ps1", bufs=2, space="PSUM") as ps1_pool,
            tc.tile_pool(name="ps2", bufs=2, space="PSUM") as ps2_pool,
        ):
            a_sb = a_pool.tile([P, NCH, WB], f16)
            # A (156KB, band-compact) rides the SP HWDGE queue: SP clears
            # the start barrier ~2us before GpSimd's first SWDGE ucode call,
            # and the transfer is long done before the first output DMA
            # needs the queue.
            nc.sync.dma_start(a_sb[:], a[:])
            # Dummy matmul consuming a_sb: PE observes the a_sb DMA semaphore
            # here, so real matmuls only ever wait on their img DMA.
            warm = ps2_pool.tile([P, 2, W], f32, tag="p2")
            nc.tensor.matmul(warm[0:2, 0, 0:2], a_sb[:, 0, 0:2], a_sb[:, 0, 0:2],
                             start=True, stop=True)
            # PSUM->SBUF copy engines, alternated so DVE and Activation each
            # carry half (Pool/GpSimd cannot access PSUM).  Each copy drains
            # a 2-bank PSUM tile (two matmul groups) in one instruction.
            def copy(which, dst, src):
                if which == 0:
                    nc.vector.tensor_copy(dst, src)
                else:
                    nc.scalar.copy(dst, src)

            def pass1(i, img, mid):
                for jh in range(2):           # pairs of output w-chunks
                    p1 = ps1_pool.tile([P, 2, H], f32)
                    for jj in range(2):
                        j = 2 * jh + jj       # output w-chunk (stationary cols)
                        for c in range(NCH):  # contraction h-chunk
                            n0, n1 = NR[c]
                            nc.tensor.matmul(
                                p1[:, jj, n0:n1],
                                img[:, c, j * P:(j + 1) * P],
                                a_sb[:, c, 0:n1 - n0],
                                start=(c == 0),
                                stop=(c == NCH - 1),
                            )
                    copy((0, 1)[jh], mid[:, 2 * jh:2 * jh + 2, :], p1[:])

            yrh = y.rearrange("i p (hh c) w -> i hh p c w", hh=2)

            def pass2(i, mid):
                out_sb = out_pool.tile([P, NCH, W], f16)
                for ih in range(2):           # pairs of output h'-chunks
                    p2 = ps2_pool.tile([P, 2, W], f32, tag="p2")
                    for jj in range(2):
                        ii = 2 * ih + jj      # output h'-chunk
                        for j in range(NCH):  # contraction w-chunk
                            n0, n1 = NR[j]
                            nc.tensor.matmul(
                                p2[:, jj, n0:n1],
                                mid[:, j, ii * P:(ii + 1) * P],
                                a_sb[:, j, 0:n1 - n0],
                                start=(j == 0),
                                stop=(j == NCH - 1),
                            )
                    copy((1, 0)[ih], out_sb[:, 2 * ih:2 * ih + 2, :], p2[:])
                    # Half-image output DMA right after the pair's copy, on
                    # the SP HWDGE queue (separate from the input stream on
                    # the GpSimd SWDGE queue): smooths the write stream and
                    # shortens the end-of-kernel drain.
                    nc.sync.dma_start(yrh[i, ih], out_sb[:, 2 * ih:2 * ih + 2, :])

            # Software pipeline: pass2(i-1) is emitted after pass1(i), so the
            # PE rolls straight from pass1(i) into pass2(i-1) (whose mid
            # copies completed during pass1(i)) with no dependency bubble.
            mids = [None] * IMGS
            for i in range(IMGS):
                img = img_pool.tile([P, NCH, W], f16)
                # Image 0 rides the SP queue with A (starts ~2us earlier);
                # the steady-state input stream stays on the GpSimd SWDGE
                # queue, separate from the output stream on SP.
                (nc.sync if i == 0 else nc.gpsimd).dma_start(img[:], xr[i])
                mids[i] = mid_pool.tile([P, NCH, H], f16, name=f"mid{i}", tag="mid")
                pass1(i, img, mids[i])
                if i > 0:
                    pass2(i - 1, mids[i - 1])
            pass2(IMGS - 1, mids[IMGS - 1])

    nc.compile()
    _NC_CACHE["nc"] = nc
    return nc


def _make_in_maps(x: np.ndarray, sigma: float):
    Afull = band_matrix(float(sigma))
    A = np.zeros((P, NCH, WB), np.float16)
    for c in range(NCH):
        n0, n1 = NR[c]
        A[:, c, 0:n1 - n0] = Afull[c * P:(c + 1) * P, n0:n1]
    # partition-major device layout: [i, p, c, w] with image row h = c*128+p
    xs = (
        x.reshape(BATCH * CH, NCH, P, W)
        .transpose(0, 2, 1, 3)
        .astype(np.float16)
    )
    in_maps = []
    for core in range(NCORES):
        shard = np.ascontiguousarray(xs[core * IMGS:(core + 1) * IMGS])
        in_maps.append({"x": shard, "a": A})
    return in_maps


def run_spmd(x: np.ndarray, sigma: float, **kw):
    """Run on 8 cores; returns (full_output, BassKernelResults)."""
    from concourse.bass_utils import run_bass_kernel_spmd

    nc = _build_nc()
    in_maps = _make_in_maps(x, sigma)
    br = run_bass_kernel_spmd(nc, in_maps, list(range(NCORES)), **kw)
    # undo the partition-major layout: [i, p, c, w] -> [i, (c p), w]
    out = np.concatenate(
        [
            r["y"].transpose(0, 2, 1, 3).reshape(B_PER_CORE, CH, H, W)
            for r in br.results
        ],
        axis=0,
    )
    return np.ascontiguousarray(out.astype(np.float32)), br


def kernel(x: np.ndarray, sigma: np.ndarray) -> np.ndarray:
    out, _ = run_spmd(np.asarray(x), float(np.asarray(sigma)))
    return out


# revision 53
# speedup vs baseline: 1.0773x; 1.0773x over previous
"""Trainium2 Bass kernel: depthwise 19x19 Gaussian blur (sigma learnable scalar).

Math: the normalized 2D Gaussian kernel is exactly separable, K2 = outer(t, t)
with t = g / sum(g), g[i] = exp(-(i-9)^2 / (2 sigma^2)).  Each 1D conv (SAME,
zero pad) is a banded-matrix product.  On the TensorEngine, a matmul
out[m,n] = sum_k lhsT[k,m] * rhs[k,n] with lhsT = image chunk (stationary) and
rhs = banded matrix A (A[h,h'] = t[h-h'+9]) computes the vertical conv AND
transposes, so two chained passes land back in the original orientation with
zero explicit transposes:
  pass1: y1T[w,h'] = sum_h img[h,w]  * A[h,h']   (img tiles stationary)
  pass2: out[h',w']= sum_w y1T[w,h'] * A[w,w']   (y1T tiles stationary)
Bands are trimmed per 128-row k-chunk so each matmul streams only ~140-152
columns; the 4 chunk contributions accumulate into one PSUM bank per output
chunk (start=True only on the first).

All HBM I/O and matmul operands are fp16 (PSUM accumulation stays fp32):
fp16 matmuls stream 1 cycle/row vs fp32's 4, and the DMA bytes halve.  The
output tolerance (rel err < 2e-2) leaves ample room (~4e-4 realized).
PSUM->SBUF copies alternate between DVE and Activation (2-bank tiles, one
copy per two matmul groups); the input stream rides the GpSimd SWDGE queue
and the output stream the SP HWDGE queue.  x/y use a partition-major DRAM
layout (host permute) for 2KB DMA packets; A is stored band-compact.

Sharding: pure data parallel, 2 batches (32 images of 512x512) per core
across 8 cores.
"""

import sys

for _p in ("/opt/trn_rl_repo", "/root/.axon_site/_ro/trn_rl_repo"):
    if _p not in sys.path:
        sys.path.append(_p)

import numpy as np

H = 512
W = 512
KS = 19
HALF = KS // 2
CH = 16
BATCH = 16
NCORES = 8
B_PER_CORE = BATCH // NCORES          # 2
IMGS = B_PER_CORE * CH                # 32 images per core
P = 128
NCH = H // P                          # 4 chunks of 128 rows
# Per k-chunk c the nonzero band of A covers columns [128c-9, 128c+136];
# widened to 8B-aligned boundaries (multiples of 4 fp16 elems), clipped
# to [0, 512).
NR = [(0, 140), (116, 268), (244, 396), (372, 512)]
WB = max(n1 - n0 for n0, n1 in NR)            # 152: compact band storage


def _taps(sigma: float) -> np.ndarray:
    coords = np.arange(-HALF, HALF + 1, dtype=np.float64)
    g = np.exp(-(coords ** 2) / (2.0 * float(sigma) ** 2))
    return g / g.sum()


def band_matrix(sigma: float) -> np.ndarray:
    """A[i, j] = t[i - j + HALF] for |i-j| <= HALF, else 0.  (512, 512)."""
    t = _taps(sigma)
    A = np.zeros((H, H), np.float64)
    idx = np.arange(H)
    for d in range(-HALF, HALF + 1):
        sel = idx[(idx + d >= 0) & (idx + d < H)]
        A[sel, sel + d] = t[HALF - d]
    return A.astype(np.float16)


_NC_CACHE = {}


def _build_nc():
    if "nc" in _NC_CACHE:
        return _NC_CACHE["nc"]
    from concourse import bacc, tile, mybir

    f16 = mybir.dt.float16
    f32 = mybir.dt.float32
    nc = bacc.Bacc(None)
    # x and y live in DRAM partition-major ([img, p, c, w] with h = c*128+p,
    # permuted on the host): every SBUF partition's 4KB is contiguous in
    # DRAM, so the DGE emits large packets instead of 1KB row packets.
    x = nc.declare_dram_parameter("x", [IMGS, P, NCH, W], f16, isOutput=False)
    # A stored band-compact ([p, c, 0:n1-n0] = A[c*128+p, n0:n1]): 156KB
    # instead of 524KB, so it lands in SBUF well before the first image.
    a = nc.declare_dram_parameter("a", [P, NCH, WB], f16, isOutput=False)
    y = nc.declare_dram_parameter("y", [IMGS, P, NCH, W], f16, isOutput=True)

    xr = x                                          # [IMGS, 128, 4, 512]

    with tile.TileContext(nc) as tc:
        with (
            tc.tile_pool(name="aco", bufs=1) as a_pool,
            tc.tile_pool(name="img", bufs=6) as img_pool,
            tc.tile_pool(name="mid", bufs=4) as mid_pool,
            tc.tile_pool(name="ost", bufs=6) as out_pool,
            tc.tile_pool(name="ps1", bufs=2, space="PSUM") as ps1_pool,
            tc.tile_pool(name="ps2", bufs=2, space="PSUM") as ps2_pool,
        ):
            a_sb = a_pool.tile([P, NCH, WB], f16)
            # Dynamic (SWDGE) DMA for A on the input queue; band-compact A
            # (156KB) lands well before the first image.  (Putting A or any
            # input on the SP queue — even just A+img0 — delays the output
            # stream ~10us: measured net regression, twice.)
            nc.gpsimd.dma_start(a_sb[:], a[:])
            # Dummy matmul consuming a_sb: PE observes the a_sb DMA semaphore
            # here, so real matmuls only ever wait on their img DMA.
            warm = ps2_pool.tile([P, 2, W], f32, tag="p2")
            nc.tensor.matmul(warm[0:2, 0, 0:2], a_sb[:, 0, 0:2], a_sb[:, 0, 0:2],
                             start=True, stop=True)
            # PSUM->SBUF copy engines, alternated so DVE and Activation each
            # carry half (Pool/GpSimd cannot access PSUM).  Each copy drains
            # a 2-bank PSUM tile (two matmul groups) in one instruction.
            def copy(which, dst, src):
                if which == 0:
                    nc.vector.tensor_copy(dst, src)
                else:
                    nc.scalar.copy(dst, src)

            def pass1(i, img, mid):
                for jh in range(2):           # pairs of output w-chunks
                    p1 = ps1_pool.tile([P, 2, H], f32)
                    for jj in range(2):
                        j = 2 * jh + jj       # output w-chunk (stationary cols)
                        for c in range(NCH):  # contraction h-chunk
                            n0, n1 = NR[c]
                            nc.tensor.matmul(
                                p1[:, jj, n0:n1],
                                img[:, c, j * P:(j + 1) * P],
                                a_sb[:, c, 0:n1 - n0],
                                start=(c == 0),
                                stop=(c == NCH - 1),
                            )
                    copy((0, 1)[jh], mid[:, 2 * jh:2 * jh + 2, :], p1[:])

            yrh = y.rearrange("i p (hh c) w -> i hh p c w", hh=2)

            def pass2(i, mid):
                out_sb = out_pool.tile([P, NCH, W], f16)
                for ih in range(2):           # pairs of output h'-chunks
                    p2 = ps2_pool.tile([P, 2, W], f32, tag="p2")
                    for jj in range(2):
                        ii = 2 * ih + jj      # output h'-chunk
                        for j in range(NCH):  # contraction w-chunk
                            n0, n1 = NR[j]
                            nc.tensor.matmul(
                                p2[:, jj, n0:n1],
                                mid[:, j, ii * P:(ii + 1) * P],
                                a_sb[:, j, 0:n1 - n0],
                                start=(j == 0),
                                stop=(j == NCH - 1),
                            )
                    copy((1, 0)[ih], out_sb[:, 2 * ih:2 * ih + 2, :], p2[:])
                    # Half-image output DMA right after the pair's copy, on
                    # the SP HWDGE queue (separate from the input stream on
                    # the GpSimd SWDGE queue): smooths the write stream and
                    # shortens the end-of-kernel drain.
                    nc.sync.dma_start(yrh[i, ih], out_sb[:, 2 * ih:2 * ih + 2, :])

            # Software pipeline: pass2(i-1) is emitted after pass1(i), so the
            # PE rolls straight from pass1(i) into pass2(i-1) (whose mid
            # copies completed during pass1(i)) with no dependency bubble.
            mids = [None] * IMGS
            for i in range(IMGS):
                img = img_pool.tile([P, NCH, W], f16)
                # Input stream on the GpSimd SWDGE queue, separate from the
                # output stream on the SP HWDGE queue.
                nc.gpsimd.dma_start(img[:], xr[i])
                mids[i] = mid_pool.tile([P, NCH, H], f16, name=f"mid{i}", tag="mid")
                pass1(i, img, mids[i])
                if i > 0:
                    pass2(i - 1, mids[i - 1])
            pass2(IMGS - 1, mids[IMGS - 1])

    nc.compile()
    _NC_CACHE["nc"] = nc
    return nc


def _make_in_maps(x: np.ndarray, sigma: float):
    Afull = band_matrix(float(sigma))
    A = np.zeros((P, NCH, WB), np.float16)
    for c in range(NCH):
        n0, n1 = NR[c]
        A[:, c, 0:n1 - n0] = Afull[c * P:(c + 1) * P, n0:n1]
    # partition-major device layout: [i, p, c, w] with image row h = c*128+p
    xs = (
        x.reshape(BATCH * CH, NCH, P, W)
        .transpose(0, 2, 1, 3)
        .astype(np.float16)
    )
    in_maps = []
    for core in range(NCORES):
        shard = np.ascontiguousarray(xs[core * IMGS:(core + 1) * IMGS])
        in_maps.append({"x": shard, "a": A})
    return in_maps


def run_spmd(x: np.ndarray, sigma: float, **kw):
    """Run on 8 cores; returns (full_output, BassKernelResults)."""
    from concourse.bass_utils import run_bass_kernel_spmd

    nc = _build_nc()
    in_maps = _make_in_maps(x, sigma)
    br = run_bass_kernel_spmd(nc, in_maps, list(range(NCORES)), **kw)
    # undo the partition-major layout: [i, p, c, w] -> [i, (c p), w]
    out = np.concatenate(
        [
            r["y"].transpose(0, 2, 1, 3).reshape(B_PER_CORE, CH, H, W)
            for r in br.results
        ],
        axis=0,
    )
    return np.ascontiguousarray(out.astype(np.float32)), br


def kernel(x: np.ndarray, sigma: np.ndarray) -> np.ndarray:
    out, _ = run_spmd(np.asarray(x), float(np.asarray(sigma)))
    return out
